# revision 1
# baseline (speedup 1.0000x reference)
"""Trainium2 Bass kernel for a GPT-J-style (parallel-residual) decoder layer.

Problem: B=2, S=2048, D=1024, H=16 heads x 64, rotary_dim=16, FF=4096, causal.

Sharding (8 NeuronCores): data-parallel over batch (2) x tensor-parallel over
heads/FFN (4).  Core c handles batch c//4 and TP rank r=c%4: heads 4r..4r+3
(256 of the 1024 attention dims), FFN rows 1024r..1024r+1024.
LayerNorm affine params are folded into the weights on the host, so the device
computes a single normalized activation xhat shared by attention and FFN.
Each core returns partial^T = (attn_partial + ffn_partial)^T in [D, S]; the
host sums the 4 TP partials per batch and adds x + b_o + b2.

v2 layout: x arrives pre-transposed [D, S] so LayerNorm stats run as ones-
matmuls on the PE (warming it immediately) and xhat^T is normalized in place
in SBUF -- no DRAM staging or DMA transposes for activations.  FFN2 and the
attention output projection accumulate into the same PSUM bank per output
block so no partial is staged to DRAM.  Attention processes 512-query blocks;
head pairs' score matmuls interleave at PE row groups (0,0)/(64,0) so they
run concurrently (K=64 each).
"""

import numpy as np
import ml_dtypes

import concourse.bass as bass
import concourse.mybir as mybir
import concourse.tile as tile
import concourse.bass_utils as bass_utils
from concourse import bacc
from concourse.bass import ds, ts

B, S, D = 2, 2048, 1024
H, HD = 16, 64
ROT, RH = 16, 8
FF = 4096
EPS = 1e-5
P = 128
NT = S // P            # 16 sequence tiles
DC = D // P            # 8 model-dim chunks
NG = 4                 # 512-column groups of S
GW = S // NG           # 512
NH = 4                 # heads per core
DSH = NH * HD          # 256 attention dims per core
FSH = FF // 4          # 1024 FFN rows per core
NCORES = 8

F32 = mybir.dt.float32
BF16 = mybir.dt.bfloat16
AF = mybir.ActivationFunctionType
ALU = mybir.AluOpType
bf16 = ml_dtypes.bfloat16


def _body(tc, aps, gelu_func):
    nc = tc.nc
    xt_d = aps["xt"]
    wqkv_d = aps["wqkv"]
    bqkv_d = aps["bqkv"]
    wo_d = aps["wo"]
    w1_d = aps["w1"]
    b1_d = aps["b1p"]
    w2_d = aps["w2"]
    cos_d = aps["cosr"]
    sin_d = aps["sinr"]
    mask_d = aps["maskd"]
    out_d = aps["outp"].rearrange("(c p) s -> c p s", p=P)   # [8, 128, 2048]

    with (
        tc.tile_pool(name="const", bufs=1) as const,
        tc.tile_pool(name="big", bufs=1) as big,
        tc.tile_pool(name="sqp", bufs=2) as sqp,
        tc.tile_pool(name="rowp", bufs=1) as rowp,
        tc.tile_pool(name="xhp", bufs=4) as xhp,
        tc.tile_pool(name="rotp", bufs=3) as rotp,
        tc.tile_pool(name="wstp", bufs=3) as wstp,
        tc.tile_pool(name="ptp", bufs=3) as ptp,
        tc.tile_pool(name="sump", bufs=2) as sump,
        tc.tile_pool(name="obp", bufs=3) as obp,
    ):
        # ---- persistent SBUF (loads for QKV/FFN weights are emitted after
        # the xt chunk loads so stats aren't delayed) ----
        wqkv_sb = const.tile([P, DC, 3 * DSH], BF16)
        bqkv_sb = const.tile([1, 3 * DSH], BF16)
        wo_sb = const.tile([P, 2, D], BF16)
        b1_sb = const.tile([P, DC], F32)
        w2_sb = const.tile([P, DC, DC, P], BF16)
        cos_sb = const.tile([P, NT, RH], BF16)
        sin_sb = const.tile([P, NT, RH], BF16)
        mask_sb = const.tile([P, P], BF16)
        ones_hd = const.tile([1, HD], BF16)
        nc.vector.memset(ones_hd[:], 1.0)
        ones_p = const.tile([1, P], BF16)
        nc.vector.memset(ones_p[:], 1.0)
        oneK = const.tile([P, 1], BF16)
        nc.vector.memset(oneK[:], 1.0 / D)
        gate = const.tile([P, 1], F32)
        eps_sb = const.tile([1, 1], F32)
        nc.vector.memset(eps_sb[:], EPS)

        # xt doubles as xhatT after the in-place normalize
        xt = big.tile([P, DC, S], BF16)             # x^T  [d, s] chunks
        qk = big.tile([P, NT, 2 * DSH], BF16)       # q,k token-major
        vp = big.tile([P, NT, NH, HD + 2], BF16)    # v token-major + ones col
        qe = big.tile([P, 2, S], BF16)              # q e-major
        ke = big.tile([P, 2, S], BF16)              # k e-major
        ot = big.tile([P, 2, S], BF16)              # attn out (normalized), e-major
        hid = big.tile([P, DC, S], BF16)            # ffn hidden, f-major
        rstdB = big.tile([P, S], BF16)              # rstd broadcast rows
        mrsB = big.tile([P, S], BF16)               # mu*rstd broadcast rows

        nc.vector.memset(vp[:, :, :, HD:HD + 1], 1.0)

        # DRAM staging for the q/k bf16 transpose (token-major -> e-major)
        stg = tc.alloc_tile_pool(name="stg", bufs=1, space="DRAM")
        qk_dram = stg.tile([S, 2 * DSH], BF16)

        # ---- Stage A: LayerNorm stats via ones-matmuls + in-place xhat^T ----
        with tc.tile_pool(name="stps", bufs=1, space="PSUM") as stps:
            st = [stps.tile([33, GW], F32, name=f"st{g}") for g in range(NG)]
            for c in range(DC):
                (nc.sync if c % 2 == 0 else nc.scalar).dma_start(
                    xt[:, c, :], xt_d[:, c, :])
                sq = sqp.tile([P, S], BF16, tag="sq")
                nc.scalar.square(sq[:], xt[:, c, :])
                for g in range(NG):
                    nc.tensor.matmul(st[g][0:1, :], lhsT=oneK[:],
                                     rhs=xt[:, c, ts(g, GW)],
                                     start=(c == 0), stop=(c == DC - 1))
                    nc.tensor.matmul(st[g][32:33, :], lhsT=oneK[:],
                                     rhs=sq[:, ts(g, GW)],
                                     start=(c == 0), stop=(c == DC - 1))
            # weight loads queue up behind the xt chunks
            nc.sync.dma_start(wqkv_sb[:], wqkv_d)
            nc.scalar.dma_start(bqkv_sb[:], bqkv_d)
            nc.scalar.dma_start(cos_sb[:], cos_d)
            nc.scalar.dma_start(sin_sb[:], sin_d)
            nc.scalar.dma_start(mask_sb[:], mask_d)
            nc.sync.dma_start(wo_sb[:], wo_d)
            nc.scalar.dma_start(b1_sb[:], b1_d)
            with tc.tile_pool(name="bcps", bufs=2, space="PSUM") as bcps:
                for g in range(NG):
                    musq = rowp.tile([1, GW], F32, tag="mu", name=f"mu{g}")
                    nc.scalar.square(musq[:], st[g][0:1, :])
                    var = rowp.tile([1, GW], F32, tag="var", name=f"var{g}")
                    nc.vector.tensor_tensor(out=var[:], in0=st[g][32:33, :],
                                            in1=musq[:], op=ALU.subtract)
                    std = rowp.tile([1, GW], F32, tag="std", name=f"std{g}")
                    nc.scalar.activation(std[:], var[:], AF.Sqrt,
                                         bias=eps_sb[:])
                    rstd = rowp.tile([1, GW], F32, tag="rsd", name=f"rsd{g}")
                    nc.vector.reciprocal_approx_fast(out=rstd[:], in_=std[:])
                    rstd_bf = rowp.tile([1, GW], BF16, tag="rsb",
                                        name=f"rsb{g}")
                    nc.vector.tensor_copy(out=rstd_bf[:], in_=rstd[:])
                    mrs_bf = rowp.tile([1, GW], BF16, tag="mrb",
                                       name=f"mrb{g}")
                    nc.vector.tensor_tensor(out=mrs_bf[:], in0=st[g][0:1, :],
                                            in1=rstd[:], op=ALU.mult)
                    bc1 = bcps.tile([P, GW], F32, tag="bc")
                    nc.tensor.matmul(bc1[:], lhsT=ones_p[:], rhs=rstd_bf[:],
                                     start=True, stop=True)
                    nc.scalar.copy(out=rstdB[:, ts(g, GW)], in_=bc1[:])
                    bc2 = bcps.tile([P, GW], F32, tag="bc")
                    nc.tensor.matmul(bc2[:], lhsT=ones_p[:], rhs=mrs_bf[:],
                                     start=True, stop=True)
                    nc.scalar.copy(out=mrsB[:, ts(g, GW)], in_=bc2[:])
                    # in-place normalize: xt <- xt*rstd - mu*rstd  (= xhat^T)
                    for c in range(DC):
                        sl = xt[:, c, ts(g, GW)]
                        nc.vector.tensor_tensor(out=sl, in0=sl,
                                                in1=rstdB[:, ts(g, GW)],
                                                op=ALU.mult)
                        nc.vector.tensor_tensor(out=sl, in0=sl,
                                                in1=mrsB[:, ts(g, GW)],
                                                op=ALU.subtract)

        # ---- Stage B: QKV projection (group-ordered), rotary, q/k transpose,
        # FFN-1 + GELU ----
        with (
            tc.tile_pool(name="qaps", bufs=2, space="PSUM") as qaps,
            tc.tile_pool(name="qbps", bufs=2, space="PSUM") as qbps,
            tc.tile_pool(name="ff1ps", bufs=4, space="PSUM") as ff1ps,
        ):
            for g in range(NG):
                for t in range(4 * g, 4 * g + 4):
                    psa = qaps.tile([P, 512], F32, tag="qa")
                    psb = qbps.tile([P, 256], F32, tag="qb")
                    for c in range(DC):
                        l = xt[:, c, ts(t, P)]
                        nc.tensor.matmul(psa[:], lhsT=l, rhs=wqkv_sb[:, c, 0:512],
                                         start=(c == 0), stop=False)
                        nc.tensor.matmul(psb[:], lhsT=l, rhs=wqkv_sb[:, c, 512:768],
                                         start=(c == 0), stop=False)
                    # bias add on the PE (K=1 ones row), PSUM drain on ScalarE
                    nc.tensor.matmul(psa[:], lhsT=ones_p[:],
                                     rhs=bqkv_sb[:, 0:512],
                                     start=False, stop=True)
                    nc.tensor.matmul(psb[:], lhsT=ones_p[:],
                                     rhs=bqkv_sb[:, 512:768],
                                     start=False, stop=True)
                    nc.scalar.copy(out=qk[:, t, :], in_=psa[:])
                    nc.scalar.copy(
                        out=vp[:, t, :, 0:HD],
                        in_=psb[:].rearrange("p (h e) -> p h e", h=NH))

                if g % 2 == 1:
                    # rotary + qk_dram writes per 8-tile half so the q/k
                    # transpose isn't gated on the whole QKV phase
                    hf = g // 2
                    tsl = ds(8 * hf, 8)
                    cosb = cos_sb[:, tsl, :].unsqueeze(2).to_broadcast(
                        [P, 8, NH, RH])
                    sinb = sin_sb[:, tsl, :].unsqueeze(2).to_broadcast(
                        [P, 8, NH, RH])
                    for part in range(2):   # 0: q, 1: k
                        sl = qk[:, tsl, ds(DSH * part, DSH)].rearrange(
                            "p t (h e) -> p t h e", h=NH)
                        x1 = sl[:, :, :, 0:RH]
                        x2 = sl[:, :, :, RH:ROT]
                        t1 = rotp.tile([P, 8, NH, RH], BF16, tag="rt",
                                       name=f"t1_{g}_{part}")
                        t2 = rotp.tile([P, 8, NH, RH], BF16, tag="rt",
                                       name=f"t2_{g}_{part}")
                        t3 = rotp.tile([P, 8, NH, RH], BF16, tag="rt",
                                       name=f"t3_{g}_{part}")
                        nc.vector.tensor_tensor(out=t1[:], in0=x1, in1=cosb,
                                                op=ALU.mult)
                        nc.vector.tensor_tensor(out=t2[:], in0=x2, in1=sinb,
                                                op=ALU.mult)
                        nc.vector.tensor_tensor(out=t1[:], in0=t1[:], in1=t2[:],
                                                op=ALU.subtract)
                        nc.vector.tensor_tensor(out=t2[:], in0=x1, in1=sinb,
                                                op=ALU.mult)
                        nc.vector.tensor_tensor(out=t3[:], in0=x2, in1=cosb,
                                                op=ALU.mult)
                        nc.vector.tensor_tensor(out=t2[:], in0=t2[:], in1=t3[:],
                                                op=ALU.add)
                        nc.vector.tensor_copy(out=x1, in_=t1[:])
                        nc.vector.tensor_copy(out=x2, in_=t2[:])
                    for t in range(8 * hf, 8 * hf + 8):
                        (nc.sync if t % 2 == 0 else nc.scalar).dma_start(
                            qk_dram[ts(t, P), :], qk[:, t, :])

            # transpose q, k to e-major (via DRAM staging)
            for c in range(2):
                nc.sync.dma_start_transpose(qe[:, c, :],
                                            qk_dram[:, ds(P * c, P)])
                nc.scalar.dma_start_transpose(ke[:, c, :],
                                              qk_dram[:, ds(DSH + P * c, P)])

            # FFN first matmul + GELU (w2 streams in under this phase)
            nc.scalar.dma_start(w2_sb[:], w2_d)
            for ft in range(DC):
                w1t = wstp.tile([P, DC, P], BF16, tag="wst", name=f"w1t_{ft}")
                nc.scalar.dma_start(w1t[:], w1_d[:, ft])
                pss = [ff1ps.tile([P, 512], F32, tag="ff1",
                                  name=f"ff1_{ft}_{i}") for i in range(4)]
                for c in range(DC):
                    for sc in range(4):
                        nc.tensor.matmul(
                            pss[sc][:], lhsT=w1t[:, c, :],
                            rhs=xt[:, c, ts(sc, GW)],
                            start=(c == 0), stop=(c == DC - 1))
                for sc in range(4):
                    nc.scalar.activation(hid[:, ft, ts(sc, GW)],
                                         pss[sc][:], gelu_func,
                                         bias=b1_sb[:, ft:ft + 1])

        # gate: forces every Exp to wait until the last FFN1 GELUs are done so
        # the ACT table never alternates between Gelu and Exp
        gpre = sump.tile([P, 1], F32, tag="gp")
        nc.vector.tensor_reduce(gpre[:], hid[:, DC - 1, :],
                                axis=mybir.AxisListType.X, op=ALU.max)
        nc.vector.tensor_scalar(out=gate[:], in0=gpre[:], scalar1=0.0,
                                scalar2=None, op0=ALU.mult)

        # ---- Stage C: attention by 512-query blocks (head-pair packed
        # scores), fused FFN2 + W_o accumulation per output block ----
        maskb = mask_sb[:].unsqueeze(1).to_broadcast([P, 2, P])
        with (
            tc.tile_pool(name="scps", bufs=2, space="PSUM") as scps,
            tc.tile_pool(name="ovps", bufs=2, space="PSUM") as ovps,
            tc.tile_pool(name="f2ps", bufs=2, space="PSUM") as f2ps,
        ):
            def emit_fused(fsc, et):
                # fused FFN2 + W_o accumulation for output block (et, fsc)
                po = f2ps.tile([P, GW], F32, tag="f2", name=f"f2_{fsc}_{et}")
                for c in range(DC):
                    nc.tensor.matmul(po[:], lhsT=w2_sb[:, et, c, :],
                                     rhs=hid[:, c, ts(fsc, GW)],
                                     start=(c == 0), stop=False)
                for c in range(2):
                    nc.tensor.matmul(po[:], lhsT=wo_sb[:, c, ts(et, P)],
                                     rhs=ot[:, c, ts(fsc, GW)],
                                     start=False, stop=(c == 1))
                ob = obp.tile([P, GW], BF16, tag="ob", name=f"ob_{fsc}_{et}")
                nc.vector.tensor_copy(out=ob[:], in_=po[:])
                oeng = nc.sync if et % 2 == 0 else nc.scalar
                oeng.dma_start(out_d[et][:, ts(fsc, GW)], ob[:])

            for sc in range(NG):
                # fused blocks of the previous query block interleave into
                # this block's attention steps so the PE never waits on Exp
                todo = list(range(DC)) if sc > 0 else []
                nsteps = 2 * (4 * sc + 4)
                per = max(1, nsteps // DC)
                step = 0
                for pair in range(2):       # heads (2*pair, 2*pair+1)
                    ov = [ovps.tile([P, GW], F32, tag="ov",
                                    name=f"ov_{sc}_{pair}_{hl}")
                          for hl in range(2)]
                    nlast = 4 * sc + 3
                    for i in range(nlast + 1):
                        qoff = max(GW * sc, P * i)
                        w = GW * (sc + 1) - qoff
                        ps = scps.tile([P, 2, GW], F32, tag="sc",
                                       name=f"sc_{sc}_{pair}_{i}")
                        for hl in range(2):
                            b0 = HD * hl
                            nc.tensor.matmul(
                                ps[:, hl, 0:w],
                                lhsT=ke[b0:b0 + HD, pair, ts(i, P)],
                                rhs=qe[b0:b0 + HD, pair, ds(qoff, w)],
                                start=True, stop=True)
                        pt = ptp.tile([P, 2, GW], BF16, tag="pt",
                                      name=f"pt_{sc}_{pair}_{i}")
                        nc.scalar.activation(pt[:, :, 0:w], ps[:, :, 0:w],
                                             AF.Exp, scale=0.125,
                                             bias=gate[:])
                        if P * i >= GW * sc:     # diagonal block: mask
                            nc.vector.tensor_tensor(
                                out=pt[:, :, 0:P], in0=pt[:, :, 0:P],
                                in1=maskb, op=ALU.mult)
                        for hl in range(2):
                            h = 2 * pair + hl
                            nc.tensor.matmul(
                                ov[hl][0:HD + 1, ds(qoff - GW * sc, w)],
                                lhsT=vp[:, i, h, 0:HD + 1],
                                rhs=pt[:, hl, 0:w],
                                start=(i == 0), stop=(i == nlast))
                        step += 1
                        if todo and step % per == 0:
                            emit_fused(sc - 1, todo.pop(0))
                    for hl in range(2):
                        b0 = HD * hl
                        dst = ot[b0:b0 + HD, pair, ts(sc, GW)]
                        # NOTE: dst copy is emitted before the sums copy so
                        # both DVE reads of this bank precede the broadcast
                        # matmul's write into partitions 64..127 (PE-W/DVE-R
                        # same-bank hazard)
                        nc.vector.tensor_copy(out=dst, in_=ov[hl][0:HD, :])
                        sume = sump.tile([1, GW], F32, tag="se")
                        nc.vector.tensor_copy(out=sume[:],
                                              in_=ov[hl][HD:HD + 1, :])
                        rinv = sump.tile([1, GW], F32, tag="ri")
                        nc.vector.reciprocal_approx_fast(out=rinv[:], in_=sume[:])
                        rinv_bf = sump.tile([1, GW], BF16, tag="rib")
                        nc.vector.tensor_copy(out=rinv_bf[:], in_=rinv[:])
                        nc.tensor.matmul(ov[hl][HD:P, :], lhsT=ones_hd[:],
                                         rhs=rinv_bf[:], start=True, stop=True)
                        nc.vector.tensor_tensor(out=dst, in0=dst,
                                                in1=ov[hl][HD:P, :],
                                                op=ALU.mult)
                for et in todo:
                    emit_fused(sc - 1, et)
            for et in range(DC):
                emit_fused(NG - 1, et)
        stg.release()


def build(gelu_func=None):
    if gelu_func is None:
        gelu_func = AF.Gelu
    nc = bacc.Bacc("TRN2", target_bir_lowering=False, debug=False,
                   enable_asserts=True, num_devices=NCORES)
    aps = {}

    def din(name, shape, dtype):
        aps[name] = nc.dram_tensor(name, list(shape), dtype,
                                   kind="ExternalInput").ap()

    din("xt", (P, DC, S), BF16)
    din("wqkv", (P, DC, 3 * DSH), BF16)
    din("bqkv", (1, 3 * DSH), BF16)
    din("wo", (P, 2, D), BF16)
    din("w1", (P, DC, DC, P), BF16)
    din("b1p", (P, DC), F32)
    din("w2", (P, DC, DC, P), BF16)
    din("cosr", (P, NT, RH), BF16)
    din("sinr", (P, NT, RH), BF16)
    din("maskd", (P, P), BF16)
    aps["outp"] = nc.dram_tensor("outp", [D, S], BF16,
                                 kind="ExternalOutput").ap()

    with tile.TileContext(nc) as tc:
        _body(tc, aps, gelu_func)
    nc.compile()
    return nc


def make_in_maps(inputs):
    x = np.asarray(inputs["x"], np.float32)
    Wqkv = np.asarray(inputs["W_qkv"], np.float32)
    b_qkv = np.asarray(inputs["b_qkv"], np.float32)
    Wo = np.asarray(inputs["W_o"], np.float32)
    ln1w = np.asarray(inputs["ln1_w"], np.float32)
    ln1b = np.asarray(inputs["ln1_b"], np.float32)
    ln2w = np.asarray(inputs["ln2_w"], np.float32)
    ln2b = np.asarray(inputs["ln2_b"], np.float32)
    W1 = np.asarray(inputs["W1"], np.float32)
    b1 = np.asarray(inputs["b1"], np.float32)
    W2 = np.asarray(inputs["W2"], np.float32)
    freqs = np.asarray(inputs["freqs_cis"], np.float32)

    cos = freqs[0, 0, :, :, 0]
    sin = freqs[0, 0, :, :, 1]
    cosr = np.ascontiguousarray(
        cos.reshape(NT, P, RH).transpose(1, 0, 2)).astype(bf16)
    sinr = np.ascontiguousarray(
        sin.reshape(NT, P, RH).transpose(1, 0, 2)).astype(bf16)
    kq = np.arange(P)
    maskd = (kq[:, None] <= kq[None, :]).astype(bf16)

    in_maps = []
    for core in range(NCORES):
        b = core // 4
        r = core % 4
        sl = slice(256 * r, 256 * r + 256)
        Ws = np.concatenate([Wqkv[0:D][sl], Wqkv[D:2 * D][sl],
                             Wqkv[2 * D:3 * D][sl]], 0)          # [768, 1024]
        bq = np.concatenate([b_qkv[0:D][sl], b_qkv[D:2 * D][sl],
                             b_qkv[2 * D:3 * D][sl]], 0)
        Wsp = Ws * ln1w[None, :]
        bqp = (bq + Ws @ ln1b).astype(np.float32)
        wqkv_l = np.ascontiguousarray(
            Wsp.T.reshape(DC, P, 3 * DSH).transpose(1, 0, 2)).astype(bf16)
        bqkv_l = np.ascontiguousarray(bqp[None, :]).astype(bf16)
        Wos = Wo[:, sl]                                           # [1024, 256]
        wo_l = np.ascontiguousarray(
            Wos.T.reshape(2, P, D).transpose(1, 0, 2)).astype(bf16)
        W1s = W1[FSH * r: FSH * (r + 1)]                          # [1024, 1024]
        W1p = W1s * ln2w[None, :]
        b1p = (b1[FSH * r: FSH * (r + 1)] + W1s @ ln2b).astype(np.float32)
        w1_l = np.ascontiguousarray(
            W1p.reshape(DC, P, DC, P).transpose(3, 0, 2, 1)).astype(bf16)
        b1_l = np.ascontiguousarray(b1p.reshape(DC, P).T).astype(np.float32)
        W2s = W2[:, FSH * r: FSH * (r + 1)]                       # [1024, 1024]
        w2_l = np.ascontiguousarray(
            W2s.reshape(DC, P, DC, P).transpose(3, 0, 2, 1)).astype(bf16)
        xt_l = np.ascontiguousarray(
            x[b].T.reshape(DC, P, S).transpose(1, 0, 2)).astype(bf16)
        in_maps.append(dict(
            xt=xt_l, wqkv=wqkv_l, bqkv=bqkv_l, wo=wo_l,
            w1=w1_l, b1p=b1_l, w2=w2_l, cosr=cosr, sinr=sinr, maskd=maskd))
    return in_maps


def gather(inputs, results):
    x = np.asarray(inputs["x"], np.float32)
    bias = (np.asarray(inputs["b_o"], np.float32)
            + np.asarray(inputs["b2"], np.float32))
    outs = [np.asarray(res["outp"], np.float32) for res in results]
    out = np.empty((B, S, D), np.float32)
    for b in range(B):
        acc = outs[4 * b] + outs[4 * b + 1] + outs[4 * b + 2] + outs[4 * b + 3]
        out[b] = x[b] + acc.T + bias[None, :]
    return out


_CACHE = {}


def kernel(**inputs):
    if "nc" not in _CACHE:
        _CACHE["nc"] = build()
    nc = _CACHE["nc"]
    in_maps = make_in_maps(inputs)
    res = bass_utils.run_bass_kernel_spmd(nc, in_maps,
                                          core_ids=list(range(NCORES)))
    return gather(inputs, res.results)

